# revision 18
# baseline (speedup 1.0000x reference)
"""Trainium2 Bass kernel for nn_ExpertPreferredRouter — v3.

Contract: kernel(**inputs) takes FULL inputs
  input_tokens [8, 8192, 1024] f32, W [4, 1024] f32, b [4] f32
and returns (token_mask [8, 8192] int32, expert_probs [8, 8192] f32).
One batch row per NeuronCore (8 cores).

Stream (per core):
  1. DMA x row tiles (1 MiB transfers, 2 HWDGE rings alternating).
  2. PE: fp32 transposes (bit-exact) of each [128,128] block -> PSUM.
  3. The PSUM->SBUF copies ARE the fp16 split: ACT writes x1T = fp16(xT),
     DVE writes x2T = fp16(xT - x1T).  x1+x2 carries ~22 mantissa bits.
  4. PE GEMM per 512-token block: the 3 fp16 products (x1w1 + x1w2 +
     x2w1) run CONCURRENTLY on PE column-groups 0/32/64 via
     tile_position, each accumulating its own [4, 512] logitsT slice
     over the 8 d-chunks.  The GEMM trails one block behind the
     transpose stream and its matmuls are interleaved between tiles to
     keep the PE HAM clock warm (transpose-mode alone does not count as
     busy).  Device logits land ~1e-6 of the fp32 reference, well
     inside the ~6.5e-6 minimum routing boundary gap of this input.
  5. Back-transpose: per 128-token tile, three accumulating plain
     matmuls (lhsT = the [4,128] slice of each column-group, rhs =
     identity) sum the groups into one [128, 4] PSUM tile for free;
     add bias, quartered softmax, then the exact bisection routing:
     XOR-lattice (only `mid` carried; c = (count<k)*span, mid' =
     (c|span_next) XOR mid) with a bf16 count path (counts <= 64 per
     lane are exact in bf16), 26 rounds per expert, stable tie-break
     via prefix scan + triangular matmul.
"""

import os
import numpy as np

B, N, D, E = 8, 8192, 1024, 4
NT = N // 128          # 64 token tiles per core
NCH = D // 128         # 8 contraction chunks
DMA_TILES = 2          # token tiles per dma_start (1 MiB transfers)
TPB = 4                # tiles per GEMM block (512 tokens)
NB = NT // TPB         # 16 blocks
CAPACITY = (0.1, 0.15, 0.25, 0.5)
KQUOTA = [int(np.floor(c * N)) for c in CAPACITY]   # [819, 1228, 2048, 4096]
LO_INIT = 0x3C000000
NITER = 26

_CACHE = {}
LAST_RUN = {}


def _stt_int_imm(nc, out, in0, imm, in1, op0, op1):
    from concourse import mybir
    eng = nc.vector
    return eng.add_instruction(mybir.InstTensorScalarPtr(
        name=eng.bass.get_next_instruction_name(),
        is_scalar_tensor_tensor=True, op0=op0, op1=op1,
        ins=[eng.lower_ap(in0),
             mybir.ImmediateValue(dtype=mybir.dt.int32, value=imm),
             eng.lower_ap(in1)],
        outs=[eng.lower_ap(out)]))


def _build():
    from contextlib import ExitStack
    from concourse import bacc, tile, mybir, masks

    F32 = mybir.dt.float32
    F16 = mybir.dt.float16
    BF16 = mybir.dt.bfloat16
    I32 = mybir.dt.int32
    ALU = mybir.AluOpType
    AX = mybir.AxisListType
    ACTF = mybir.ActivationFunctionType

    nc = bacc.Bacc("TRN2", target_bir_lowering=False, debug=False,
                   enable_asserts=False, num_devices=8)
    x_d = nc.dram_tensor("x", [N, D], F32, kind="ExternalInput").ap()
    w_d = nc.dram_tensor("w", [E, D], F32, kind="ExternalInput").ap()
    b_d = nc.dram_tensor("b", [1, E], F32, kind="ExternalInput").ap()
    tm_d = nc.dram_tensor("tm", [NT, 128], I32, kind="ExternalOutput").ap()
    ep_d = nc.dram_tensor("ep", [NT, 128], F32, kind="ExternalOutput").ap()

    with tile.TileContext(nc) as tc:
        with ExitStack() as ctx:
            consts = ctx.enter_context(tc.tile_pool(name="consts", bufs=1))
            xa_pool = ctx.enter_context(tc.tile_pool(name="xa", bufs=6))
            xs_pool = ctx.enter_context(tc.tile_pool(name="xs", bufs=3))
            xt_pool = ctx.enter_context(tc.tile_pool(name="xt", bufs=2))
            misc = ctx.enter_context(tc.tile_pool(name="misc", bufs=1))
            ps_tp = ctx.enter_context(tc.tile_pool(name="ps_tp", bufs=2, space="PSUM"))
            ps_g = ctx.enter_context(tc.tile_pool(name="ps_g", bufs=2, space="PSUM"))
            ps_b = ctx.enter_context(tc.tile_pool(name="ps_b", bufs=2, space="PSUM"))

            ident = consts.tile([128, 128], F32)
            masks.make_identity(nc, ident[:])
            ident4 = consts.tile([4, 4], F32)
            masks.make_identity(nc, ident4[:])
            ident68 = consts.tile([68, 4], F32)
            nc.vector.memset(ident68[0:4, :], 0.0)
            nc.sync.dma_start(ident68[0:4, :], ident[0:4, 0:4])
            nc.sync.dma_start(ident68[32:36, :], ident[0:4, 0:4])
            nc.sync.dma_start(ident68[64:68, :], ident[0:4, 0:4])
            ones128 = consts.tile([128, 128], F32)
            nc.gpsimd.memset(ones128[:], 1.0)
            ones_b = consts.tile([128, 128], BF16)
            nc.vector.tensor_copy(ones_b[:], ones128[:])
            ltmask = consts.tile([128, 128], F32)   # lt[q, p] = 1 iff q < p
            nc.gpsimd.memset(ltmask[:], 1.0)
            nc.gpsimd.affine_select(out=ltmask[:], in_=ltmask[:], compare_op=ALU.is_gt,
                                    fill=0.0, base=0, pattern=[[1, 128]],
                                    channel_multiplier=-1)

            # Issue the first x-tile DMAs before the W prep so the PE's
            # transpose stream starts as early as possible; W is only
            # needed one block later (first GEMM trails the stream).
            xa_tiles = {}

            def dma_group(g):
                xa_big = xa_pool.tile([128, D * DMA_TILES], F32, tag="xa")
                t = g * DMA_TILES
                src = x_d[128 * t:128 * (t + DMA_TILES), :].rearrange(
                    "(s p) f -> p s f", s=DMA_TILES)
                dst = xa_big[:].rearrange("p (s f) -> p s f", s=DMA_TILES)
                (nc.scalar if g % 2 else nc.sync).dma_start(dst, src)
                xa_tiles[g] = xa_big

            dma_group(0)
            dma_group(1)

            # W^T chunks in fp16 pair: WTc [128, 4] per chunk c
            w_nat = consts.tile([E, D], F32)
            nc.sync.dma_start(w_nat[:], w_d[:])
            WT = consts.tile([128, 4 * NCH], F32)
            for c in range(NCH):
                pw = ps_b.tile([128, E], F32, tag="small")
                nc.tensor.transpose(pw[:], w_nat[:, 128 * c:128 * (c + 1)],
                                    ident[0:E, 0:E])
                nc.vector.tensor_copy(WT[:, 4 * c:4 * c + 4], pw[:])
            W1h = consts.tile([128, 4 * NCH], F16)
            nc.vector.tensor_copy(W1h[:], WT[:])
            W2h = consts.tile([128, 4 * NCH], F16)
            nc.vector.tensor_tensor(W2h[:], WT[:], W1h[:], op=ALU.subtract)
            b_row = consts.tile([1, E], F32)
            nc.sync.dma_start(b_row[:], b_d[:])
            btile = consts.tile([128, E], F32)
            nc.gpsimd.partition_broadcast(btile[:], b_row[:])

            # ---- main stream ----
            probs = misc.tile([128, NT * E], F32)   # [p, t, e]; token = 128*t + p
            ep = misc.tile([128, NT], F32)
            keys3 = misc.tile([128, NT], F32)
            rmax = misc.tile([128, NT], F32)
            rsum = misc.tile([128, NT], F32)
            rinv = misc.tile([128, NT], F32)

            def softmax_quarter(q):
                # probs for tiles [16q, 16q+16) are final; normalize them.
                t_end = 16 * (q + 1)
                q0 = 4 * 16 * q
                tq = slice(q0, 4 * t_end)
                fq = slice(16 * q, t_end)
                pq = probs[:, tq].rearrange("p (t e) -> p t e", e=E)
                nc.vector.tensor_reduce(rmax[:, fq], pq, axis=AX.X, op=ALU.max)
                for e in range(E):
                    nc.vector.tensor_tensor(probs[:, q0 + e:4 * t_end:4],
                                            probs[:, q0 + e:4 * t_end:4],
                                            rmax[:, fq], op=ALU.subtract)
                nc.scalar.activation(probs[:, tq], probs[:, tq], ACTF.Exp)
                nc.vector.tensor_reduce(rsum[:, fq], pq, axis=AX.X, op=ALU.add)
                nc.vector.reciprocal(rinv[:, fq], rsum[:, fq])
                for e in range(E):
                    nc.vector.tensor_tensor(probs[:, q0 + e:4 * t_end:4],
                                            probs[:, q0 + e:4 * t_end:4],
                                            rinv[:, fq], op=ALU.mult)
                nc.vector.tensor_copy(ep[:, fq], probs[:, q0:4 * t_end:4])
                nc.vector.tensor_copy(keys3[:, fq], probs[:, q0 + 3:4 * t_end:4])

            pending_mms = []      # closures: GEMM matmuls of the previous block
            prev_epilogue = None  # closure: previous block's lsb/back-T/probs

            def make_block_closures(blk, x1T, x2T):
                pg = ps_g.tile([128, 512], F32, tag="pg")
                r1 = x1T[:].rearrange("p (t f) -> p t f", t=TPB)
                r2 = x2T[:].rearrange("p (t f) -> p t f", t=TPB)
                mms = []
                for c in range(NCH):
                    rhs1 = r1[:, :, 128 * c:128 * (c + 1)]
                    rhs2 = r2[:, :, 128 * c:128 * (c + 1)]
                    wc1 = W1h[:, 4 * c:4 * c + 4]
                    wc2 = W2h[:, 4 * c:4 * c + 4]
                    # three products run concurrently on PE column groups
                    for g, (wc, rhs) in enumerate(((wc1, rhs1), (wc2, rhs1),
                                                   (wc1, rhs2))):
                        mms.append(lambda wc=wc, rhs=rhs, g=g, c=c:
                                   nc.tensor.matmul(
                                       pg[32 * g:32 * g + 4, :], wc, rhs,
                                       start=(c == 0), stop=(c == NCH - 1),
                                       tile_position=(0, 32 * g)))

                def epilogue():
                    lsb = xs_pool.tile([68, 512], F32, tag="lsb")
                    nc.scalar.activation(lsb[:], pg[0:68, :], ACTF.Copy)
                    for tt in range(TPB):
                        t = blk * TPB + tt
                        pbt = ps_b.tile([128, 4], F32, tag="small")
                        for g in range(3):
                            nc.tensor.matmul(
                                pbt[:], lsb[32 * g:32 * g + 4,
                                             128 * tt:128 * (tt + 1)],
                                ident68[32 * g:32 * g + 4, :],
                                start=(g == 0), stop=(g == 2))
                        nc.vector.tensor_tensor(probs[:, 4 * t:4 * t + 4], pbt[:],
                                                btile[:], op=ALU.add)
                    if blk % 4 == 3:
                        softmax_quarter(blk // 4)
                return mms, epilogue

            for blk in range(NB):
                x1T = xt_pool.tile([128, TPB * D], F16, tag="x1T")
                x2T = xt_pool.tile([128, TPB * D], F16, tag="x2T")
                for tt in range(TPB):
                    t = blk * TPB + tt
                    g, off = divmod(t, DMA_TILES)
                    if g not in xa_tiles:
                        dma_group(g)
                    xa = xa_tiles[g][:, D * off:D * (off + 1)]
                    tp = ps_tp.tile([128, D], F32, tag="tp")
                    for c in range(NCH):
                        nc.tensor.transpose(tp[:, 128 * c:128 * (c + 1)],
                                            xa[:, 128 * c:128 * (c + 1)], ident[:])
                    # split during the PSUM->SBUF copies
                    x1s = x1T[:, D * tt:D * (tt + 1)]
                    nc.scalar.activation(x1s, tp[:], ACTF.Copy)
                    nc.vector.tensor_tensor(x2T[:, D * tt:D * (tt + 1)], tp[:],
                                            x1s, op=ALU.subtract)
                    # interleave previous block's GEMM to keep the PE warm
                    for _ in range(6):
                        if pending_mms:
                            pending_mms.pop(0)()
                if prev_epilogue is not None:
                    prev_epilogue()
                pending_mms, prev_epilogue = make_block_closures(blk, x1T, x2T)
            for f in pending_mms:
                f()
            prev_epilogue()

            # ---- routing (bisection per expert, exact) ----
            u = misc.tile([128, NT], F32)       # 1.0 while unassigned
            nc.vector.memset(u[:], 1.0)
            zer = misc.tile([128, NT], F32)
            nc.vector.memset(zer[:], 0.0)
            tm = misc.tile([128, NT], F32)
            nc.vector.memset(tm[:], 0.0)

            keys_m = misc.tile([128, NT], F32)
            lo = misc.tile([128, 1], I32)
            mid = misc.tile([128, 1], I32)
            msk = misc.tile([128, NT], F32)
            cpb = misc.tile([128, 1], BF16)
            step = misc.tile([128, 1], I32)
            mgt = misc.tile([128, NT], F32)
            cgt_p = misc.tile([128, 1], F32)
            r = misc.tile([128, 1], F32)
            eq = misc.tile([128, NT], F32)
            S = misc.tile([128, NT], F32)
            rank = misc.tile([128, NT], F32)
            tie = misc.tile([128, NT], F32)
            a3 = misc.tile([128, NT], F32)
            a2 = misc.tile([128, NT], F32)
            a1 = misc.tile([128, NT], F32)
            a_t = {3: a3, 2: a2, 1: a1}

            deferred = []
            for j in (3, 2, 1):
                kq = float(KQUOTA[j])
                if j == 3:
                    keys_f = keys3
                else:
                    keys_f = keys_m
                    nc.vector.tensor_tensor(keys_f[:], probs[:, j::4], u[:], op=ALU.mult)
                nc.vector.memset(mid[:], LO_INIT | (1 << (NITER - 1)))
                # XOR-lattice bisection: keep only `mid`.  Per round compute
                # c = (count < k) * span  (span bit to CLEAR), then
                # mid' = (mid XOR c) | span_next  emitted as one stt:
                # (c | span_next) XOR mid  -- c and span_next are disjoint
                # from each other and span_next is absent from mid, so the
                # XOR both clears the rejected bit and sets the next one.
                # After the last round theta = mid XOR c.
                for i in range(NITER):
                    span = 1 << (NITER - 1 - i)
                    with nc.allow_low_precision(reason="counts <= 64 exact in bf16"):
                        nc.vector.tensor_scalar(msk[:], keys_f[:], mid[:].bitcast(F32),
                                                0.0, op0=ALU.is_ge, op1=ALU.add,
                                                accum_out=cpb[:])
                    if i == 0:
                        # previous expert's tm/ep updates run on the DVE while
                        # the PE count round-trips
                        for f in deferred:
                            f()
                        deferred = []
                    psc = ps_b.tile([128, 1], F32, tag="small")
                    nc.tensor.matmul(psc[:], ones_b[:], cpb[:], start=True, stop=True)
                    nc.vector.tensor_scalar(step[:], psc[:], kq, float(span),
                                            op0=ALU.is_lt, op1=ALU.mult)
                    if i + 1 < NITER:
                        _stt_int_imm(nc, mid[:], step[:], 1 << (NITER - 2 - i), mid[:],
                                     ALU.bitwise_or, ALU.bitwise_xor)
                    else:
                        _stt_int_imm(nc, lo[:], step[:], 0, mid[:],
                                     ALU.bitwise_or, ALU.bitwise_xor)
                # theta = lo exactly
                nc.vector.tensor_scalar(mgt[:], keys_f[:], lo[:].bitcast(F32), 0.0,
                                        op0=ALU.is_gt, op1=ALU.add, accum_out=cgt_p[:])
                psg2 = ps_b.tile([128, 1], F32, tag="small")
                nc.tensor.matmul(psg2[:], ones128[:], cgt_p[:], start=True, stop=True)
                nc.vector.tensor_scalar(r[:], psg2[:], -1.0, kq, op0=ALU.mult,
                                        op1=ALU.add)
                nc.vector.tensor_scalar(eq[:], keys_f[:], lo[:].bitcast(F32), None,
                                        op0=ALU.is_equal)
                psC = ps_g.tile([128, NT], F32, tag="pg")
                nc.tensor.matmul(psC[:], ones128[:], eq[:], start=True, stop=True)
                nc.vector.tensor_tensor_scan(S[:], psC[:], zer[:], 0.0,
                                             op0=ALU.add, op1=ALU.add)
                nc.vector.tensor_tensor(S[:], S[:], psC[:], op=ALU.subtract)
                psT = ps_g.tile([128, NT], F32, tag="pg")
                nc.tensor.matmul(psT[:], ltmask[:], eq[:], start=True, stop=True)
                nc.vector.tensor_tensor(rank[:], S[:], psT[:], op=ALU.add)
                nc.vector.tensor_scalar(tie[:], rank[:], r[:], None, op0=ALU.is_lt)
                nc.vector.tensor_tensor(tie[:], tie[:], eq[:], op=ALU.mult)
                a = a_t[j]
                nc.vector.tensor_tensor(a[:], mgt[:], tie[:], op=ALU.add)
                if j != 1:
                    nc.vector.copy_predicated(u[:], a[:].bitcast(I32), zer[:])
                deferred.append(lambda a=a, j=j: (
                    nc.vector.scalar_tensor_tensor(tm[:], a[:], float(j), tm[:],
                                                   op0=ALU.mult, op1=ALU.add),
                    nc.vector.copy_predicated(ep[:], a[:].bitcast(I32),
                                              probs[:, j::4])))
            for f in deferred:
                f()

            # ---- outputs ----
            ptm = ps_g.tile([NT, 128], F32, tag="pg")
            nc.tensor.transpose(ptm[:], tm[:], ident[:])
            tm_out = misc.tile([NT, 128], I32)
            nc.vector.tensor_copy(tm_out[:], ptm[:])
            nc.sync.dma_start(tm_d[:], tm_out[:])
            pep = ps_g.tile([NT, 128], F32, tag="pg")
            nc.tensor.transpose(pep[:], ep[:], ident[:])
            ep_out = misc.tile([NT, 128], F32)
            nc.vector.tensor_copy(ep_out[:], pep[:])
            nc.scalar.dma_start(ep_d[:], ep_out[:])

    nc.compile()
    return nc


def kernel(input_tokens, W, b):
    from concourse import bass_utils

    if "nc" not in _CACHE:
        _CACHE["nc"] = _build()
    nc = _CACHE["nc"]

    x = np.ascontiguousarray(np.asarray(input_tokens, dtype=np.float32))
    Wf = np.ascontiguousarray(np.asarray(W, dtype=np.float32))
    bf = np.ascontiguousarray(np.asarray(b, dtype=np.float32)).reshape(1, E)
    in_maps = [{"x": x[i], "w": Wf, "b": bf} for i in range(B)]

    trace = bool(int(os.environ.get("CC_TRACE", "0")))
    res = bass_utils.run_bass_kernel_spmd(nc, in_maps, core_ids=list(range(B)),
                                          trace=trace)
    LAST_RUN["exec_time_ns"] = res.exec_time_ns
    LAST_RUN["trace"] = res.instructions_and_trace

    token_mask = np.stack([res.results[i]["tm"].reshape(N) for i in range(B)])
    expert_probs = np.stack([res.results[i]["ep"].reshape(N) for i in range(B)])
    return token_mask.astype(np.int32), expert_probs.astype(np.float32)


# revision 19
# speedup vs baseline: 1.0099x; 1.0099x over previous
"""Trainium2 Bass kernel for nn_ExpertPreferredRouter — v3.

Contract: kernel(**inputs) takes FULL inputs
  input_tokens [8, 8192, 1024] f32, W [4, 1024] f32, b [4] f32
and returns (token_mask [8, 8192] int32, expert_probs [8, 8192] f32).
One batch row per NeuronCore (8 cores).

Stream (per core):
  1. DMA x row tiles (1 MiB transfers, 2 HWDGE rings alternating).
  2. PE: fp32 transposes (bit-exact) of each [128,128] block -> PSUM.
  3. The PSUM->SBUF copies ARE the fp16 split: ACT writes x1T = fp16(xT),
     DVE writes x2T = fp16(xT - x1T).  x1+x2 carries ~22 mantissa bits.
  4. PE GEMM per 512-token block: the 3 fp16 products (x1w1 + x1w2 +
     x2w1) run CONCURRENTLY on PE column-groups 0/32/64 via
     tile_position, each accumulating its own [4, 512] logitsT slice
     over the 8 d-chunks.  The GEMM trails one block behind the
     transpose stream and its matmuls are interleaved between tiles to
     keep the PE HAM clock warm (transpose-mode alone does not count as
     busy).  Device logits land ~1e-6 of the fp32 reference, well
     inside the ~6.5e-6 minimum routing boundary gap of this input.
  5. Back-transpose: per 128-token tile, three accumulating plain
     matmuls (lhsT = the [4,128] slice of each column-group, rhs =
     identity) sum the groups into one [128, 4] PSUM tile for free;
     add bias, quartered softmax, then the exact bisection routing:
     XOR-lattice (only `mid` carried; c = (count<k)*span, mid' =
     (c|span_next) XOR mid) with a bf16 count path (counts <= 64 per
     lane are exact in bf16), 26 rounds per expert, stable tie-break
     via prefix scan + triangular matmul.
"""

import os
import numpy as np

B, N, D, E = 8, 8192, 1024, 4
NT = N // 128          # 64 token tiles per core
NCH = D // 128         # 8 contraction chunks
DMA_TILES = 2          # token tiles per dma_start (1 MiB transfers)
TPB = 4                # tiles per GEMM block (512 tokens)
NB = NT // TPB         # 16 blocks
CAPACITY = (0.1, 0.15, 0.25, 0.5)
KQUOTA = [int(np.floor(c * N)) for c in CAPACITY]   # [819, 1228, 2048, 4096]
LO_INIT = 0x3C000000
NITER = 26

_CACHE = {}
LAST_RUN = {}


def _stt_int_imm(nc, out, in0, imm, in1, op0, op1):
    from concourse import mybir
    eng = nc.vector
    return eng.add_instruction(mybir.InstTensorScalarPtr(
        name=eng.bass.get_next_instruction_name(),
        is_scalar_tensor_tensor=True, op0=op0, op1=op1,
        ins=[eng.lower_ap(in0),
             mybir.ImmediateValue(dtype=mybir.dt.int32, value=imm),
             eng.lower_ap(in1)],
        outs=[eng.lower_ap(out)]))


def _build():
    from contextlib import ExitStack
    from concourse import bacc, tile, mybir, masks

    F32 = mybir.dt.float32
    F16 = mybir.dt.float16
    BF16 = mybir.dt.bfloat16
    I32 = mybir.dt.int32
    ALU = mybir.AluOpType
    AX = mybir.AxisListType
    ACTF = mybir.ActivationFunctionType

    nc = bacc.Bacc("TRN2", target_bir_lowering=False, debug=False,
                   enable_asserts=False, num_devices=8)
    x_d = nc.dram_tensor("x", [N, D], F32, kind="ExternalInput").ap()
    w_d = nc.dram_tensor("w", [E, D], F32, kind="ExternalInput").ap()
    b_d = nc.dram_tensor("b", [1, E], F32, kind="ExternalInput").ap()
    tm_d = nc.dram_tensor("tm", [NT, 128], I32, kind="ExternalOutput").ap()
    ep_d = nc.dram_tensor("ep", [NT, 128], F32, kind="ExternalOutput").ap()

    with tile.TileContext(nc) as tc:
        with ExitStack() as ctx:
            consts = ctx.enter_context(tc.tile_pool(name="consts", bufs=1))
            xa_pool = ctx.enter_context(tc.tile_pool(name="xa", bufs=6))
            xs_pool = ctx.enter_context(tc.tile_pool(name="xs", bufs=3))
            xt_pool = ctx.enter_context(tc.tile_pool(name="xt", bufs=2))
            misc = ctx.enter_context(tc.tile_pool(name="misc", bufs=1))
            ps_tp = ctx.enter_context(tc.tile_pool(name="ps_tp", bufs=2, space="PSUM"))
            ps_g = ctx.enter_context(tc.tile_pool(name="ps_g", bufs=2, space="PSUM"))
            ps_b = ctx.enter_context(tc.tile_pool(name="ps_b", bufs=2, space="PSUM"))

            ident = consts.tile([128, 128], F32)
            masks.make_identity(nc, ident[:])
            ident4 = consts.tile([4, 4], F32)
            masks.make_identity(nc, ident4[:])
            ident68 = consts.tile([68, 4], F32)
            nc.vector.memset(ident68[0:4, :], 0.0)
            nc.sync.dma_start(ident68[0:4, :], ident[0:4, 0:4])
            nc.sync.dma_start(ident68[32:36, :], ident[0:4, 0:4])
            nc.sync.dma_start(ident68[64:68, :], ident[0:4, 0:4])
            ones128 = consts.tile([128, 128], F32)
            nc.gpsimd.memset(ones128[:], 1.0)
            ones_b = consts.tile([128, 128], BF16)
            nc.vector.tensor_copy(ones_b[:], ones128[:])
            ltmask = consts.tile([128, 128], F32)   # lt[q, p] = 1 iff q < p
            nc.gpsimd.memset(ltmask[:], 1.0)
            nc.gpsimd.affine_select(out=ltmask[:], in_=ltmask[:], compare_op=ALU.is_gt,
                                    fill=0.0, base=0, pattern=[[1, 128]],
                                    channel_multiplier=-1)

            # Issue the first x-tile DMAs before the W prep so the PE's
            # transpose stream starts as early as possible; W is only
            # needed one block later (first GEMM trails the stream).
            xa_tiles = {}

            def dma_group(g):
                xa_big = xa_pool.tile([128, D * DMA_TILES], F32, tag="xa")
                t = g * DMA_TILES
                src = x_d[128 * t:128 * (t + DMA_TILES), :].rearrange(
                    "(s p) f -> p s f", s=DMA_TILES)
                dst = xa_big[:].rearrange("p (s f) -> p s f", s=DMA_TILES)
                (nc.scalar if g % 2 else nc.sync).dma_start(dst, src)
                xa_tiles[g] = xa_big

            dma_group(0)
            dma_group(1)

            # W^T chunks in fp16 pair: WTc [128, 4] per chunk c
            w_nat = consts.tile([E, D], F32)
            nc.sync.dma_start(w_nat[:], w_d[:])
            WT = consts.tile([128, 4 * NCH], F32)
            for c in range(NCH):
                pw = ps_b.tile([128, E], F32, tag="small")
                nc.tensor.transpose(pw[:], w_nat[:, 128 * c:128 * (c + 1)],
                                    ident[0:E, 0:E])
                nc.vector.tensor_copy(WT[:, 4 * c:4 * c + 4], pw[:])
            W1h = consts.tile([128, 4 * NCH], F16)
            nc.vector.tensor_copy(W1h[:], WT[:])
            W2h = consts.tile([128, 4 * NCH], F16)
            nc.vector.tensor_tensor(W2h[:], WT[:], W1h[:], op=ALU.subtract)
            b_row = consts.tile([1, E], F32)
            nc.sync.dma_start(b_row[:], b_d[:])
            btile = consts.tile([128, E], F32)
            nc.gpsimd.partition_broadcast(btile[:], b_row[:])

            # ---- main stream ----
            probs = misc.tile([128, NT * E], F32)   # [p, t, e]; token = 128*t + p
            ep = misc.tile([128, NT], F32)
            keys3 = misc.tile([128, NT], F32)
            rmax = misc.tile([128, NT], F32)
            rsum = misc.tile([128, NT], F32)
            rinv = misc.tile([128, NT], F32)

            def softmax_quarter(q):
                # probs for tiles [16q, 16q+16) are final; normalize them.
                t_end = 16 * (q + 1)
                q0 = 4 * 16 * q
                tq = slice(q0, 4 * t_end)
                fq = slice(16 * q, t_end)
                pq = probs[:, tq].rearrange("p (t e) -> p t e", e=E)
                nc.vector.tensor_reduce(rmax[:, fq], pq, axis=AX.X, op=ALU.max)
                for e in range(E):
                    nc.vector.tensor_tensor(probs[:, q0 + e:4 * t_end:4],
                                            probs[:, q0 + e:4 * t_end:4],
                                            rmax[:, fq], op=ALU.subtract)
                nc.scalar.activation(probs[:, tq], probs[:, tq], ACTF.Exp)
                nc.vector.tensor_reduce(rsum[:, fq], pq, axis=AX.X, op=ALU.add)
                nc.vector.reciprocal(rinv[:, fq], rsum[:, fq])
                for e in range(E):
                    nc.vector.tensor_tensor(probs[:, q0 + e:4 * t_end:4],
                                            probs[:, q0 + e:4 * t_end:4],
                                            rinv[:, fq], op=ALU.mult)
                nc.vector.tensor_copy(ep[:, fq], probs[:, q0:4 * t_end:4])
                nc.vector.tensor_copy(keys3[:, fq], probs[:, q0 + 3:4 * t_end:4])

            pending_mms = []      # closures: GEMM matmuls of the previous block
            prev_epilogue = None  # closure: previous block's lsb/back-T/probs

            def make_block_closures(blk, x1T, x2T):
                pg = ps_g.tile([128, 512], F32, tag="pg")
                r1 = x1T[:].rearrange("p (t f) -> p t f", t=TPB)
                r2 = x2T[:].rearrange("p (t f) -> p t f", t=TPB)
                mms = []
                for c in range(NCH):
                    rhs1 = r1[:, :, 128 * c:128 * (c + 1)]
                    rhs2 = r2[:, :, 128 * c:128 * (c + 1)]
                    wc1 = W1h[:, 4 * c:4 * c + 4]
                    wc2 = W2h[:, 4 * c:4 * c + 4]
                    # three products run concurrently on PE column groups
                    for g, (wc, rhs) in enumerate(((wc1, rhs1), (wc2, rhs1),
                                                   (wc1, rhs2))):
                        mms.append(lambda wc=wc, rhs=rhs, g=g, c=c:
                                   nc.tensor.matmul(
                                       pg[32 * g:32 * g + 4, :], wc, rhs,
                                       start=(c == 0), stop=(c == NCH - 1),
                                       tile_position=(0, 32 * g)))

                def epilogue():
                    lsb = xs_pool.tile([68, 512], F32, tag="lsb")
                    nc.scalar.activation(lsb[:], pg[0:68, :], ACTF.Copy)
                    for tt in range(TPB):
                        t = blk * TPB + tt
                        pbt = ps_b.tile([128, 4], F32, tag="small")
                        for g in range(3):
                            nc.tensor.matmul(
                                pbt[:], lsb[32 * g:32 * g + 4,
                                             128 * tt:128 * (tt + 1)],
                                ident68[32 * g:32 * g + 4, :],
                                start=(g == 0), stop=(g == 2))
                        nc.vector.tensor_tensor(probs[:, 4 * t:4 * t + 4], pbt[:],
                                                btile[:], op=ALU.add)
                    if blk % 4 == 3:
                        softmax_quarter(blk // 4)
                return mms, epilogue

            for blk in range(NB):
                x1T = xt_pool.tile([128, TPB * D], F16, tag="x1T")
                x2T = xt_pool.tile([128, TPB * D], F16, tag="x2T")
                for tt in range(TPB):
                    t = blk * TPB + tt
                    g, off = divmod(t, DMA_TILES)
                    if g not in xa_tiles:
                        dma_group(g)
                    xa = xa_tiles[g][:, D * off:D * (off + 1)]
                    tp = ps_tp.tile([128, D], F32, tag="tp")
                    for c in range(NCH):
                        nc.tensor.transpose(tp[:, 128 * c:128 * (c + 1)],
                                            xa[:, 128 * c:128 * (c + 1)], ident[:])
                    # split during the PSUM->SBUF copies
                    x1s = x1T[:, D * tt:D * (tt + 1)]
                    nc.scalar.activation(x1s, tp[:], ACTF.Copy)
                    nc.vector.tensor_tensor(x2T[:, D * tt:D * (tt + 1)], tp[:],
                                            x1s, op=ALU.subtract)
                    # interleave previous block's GEMM to keep the PE warm
                    for _ in range(6):
                        if pending_mms:
                            pending_mms.pop(0)()
                if prev_epilogue is not None:
                    prev_epilogue()
                pending_mms, prev_epilogue = make_block_closures(blk, x1T, x2T)
            for f in pending_mms:
                f()
            prev_epilogue()

            # ---- routing (bisection per expert, exact) ----
            u = misc.tile([128, NT], F32)       # 1.0 while unassigned
            nc.vector.memset(u[:], 1.0)
            zer = misc.tile([128, NT], F32)
            nc.vector.memset(zer[:], 0.0)
            tm = misc.tile([128, NT], F32)
            nc.vector.memset(tm[:], 0.0)

            keys_m = misc.tile([128, NT], F32)
            lo = misc.tile([128, 1], I32)
            mid = misc.tile([128, 1], I32)
            msk = misc.tile([128, NT], F32)
            cpb = misc.tile([128, 1], BF16)
            step = misc.tile([128, 1], I32)
            mgt = misc.tile([128, NT], F32)
            cgt_p = misc.tile([128, 1], F32)
            r = misc.tile([128, 1], F32)
            eq = misc.tile([128, NT], BF16)
            S = misc.tile([128, NT], F32)
            rank = misc.tile([128, NT], F32)
            tie = misc.tile([128, NT], F32)
            a3 = misc.tile([128, NT], F32)
            a2 = misc.tile([128, NT], F32)
            a1 = misc.tile([128, NT], F32)
            a_t = {3: a3, 2: a2, 1: a1}

            ltmask_b = misc.tile([128, 128], BF16)
            nc.vector.tensor_copy(ltmask_b[:], ltmask[:])
            cgt_b = misc.tile([128, 1], BF16)

            deferred = []
            for j in (3, 2, 1):
                kq = float(KQUOTA[j])
                if j == 3:
                    keys_f = keys3
                else:
                    keys_f = keys_m
                    nc.vector.tensor_tensor(keys_f[:], probs[:, j::4], u[:], op=ALU.mult)
                nc.vector.memset(mid[:], LO_INIT | (1 << (NITER - 1)))
                # XOR-lattice bisection: keep only `mid`.  Per round compute
                # c = (count < k) * span  (span bit to CLEAR), then
                # mid' = (mid XOR c) | span_next  emitted as one stt:
                # (c | span_next) XOR mid  -- c and span_next are disjoint
                # from each other and span_next is absent from mid, so the
                # XOR both clears the rejected bit and sets the next one.
                # After the last round theta = mid XOR c.
                for i in range(NITER):
                    span = 1 << (NITER - 1 - i)
                    with nc.allow_low_precision(reason="counts <= 64 exact in bf16"):
                        nc.vector.tensor_scalar(msk[:], keys_f[:], mid[:].bitcast(F32),
                                                0.0, op0=ALU.is_ge, op1=ALU.add,
                                                accum_out=cpb[:])
                    if i == 0:
                        # previous expert's tm/ep updates run on the DVE while
                        # the PE count round-trips
                        for f in deferred:
                            f()
                        deferred = []
                    psc = ps_b.tile([128, 1], F32, tag="small")
                    nc.tensor.matmul(psc[:], ones_b[:], cpb[:], start=True, stop=True)
                    nc.vector.tensor_scalar(step[:], psc[:], kq, float(span),
                                            op0=ALU.is_lt, op1=ALU.mult)
                    if i + 1 < NITER:
                        _stt_int_imm(nc, mid[:], step[:], 1 << (NITER - 2 - i), mid[:],
                                     ALU.bitwise_or, ALU.bitwise_xor)
                    else:
                        _stt_int_imm(nc, lo[:], step[:], 0, mid[:],
                                     ALU.bitwise_or, ALU.bitwise_xor)
                # theta = lo exactly
                with nc.allow_low_precision(reason="counts <= 64 exact in bf16"):
                    nc.vector.tensor_scalar(mgt[:], keys_f[:], lo[:].bitcast(F32), 0.0,
                                            op0=ALU.is_gt, op1=ALU.add,
                                            accum_out=cgt_b[:])
                psg2 = ps_b.tile([128, 1], F32, tag="small")
                nc.tensor.matmul(psg2[:], ones_b[:], cgt_b[:], start=True, stop=True)
                nc.vector.tensor_scalar(r[:], psg2[:], -1.0, kq, op0=ALU.mult,
                                        op1=ALU.add)
                nc.vector.tensor_scalar(eq[:], keys_f[:], lo[:].bitcast(F32), None,
                                        op0=ALU.is_equal)
                psC = ps_g.tile([128, NT], F32, tag="pg")
                nc.tensor.matmul(psC[:], ones_b[:], eq[:], start=True, stop=True)
                nc.vector.tensor_tensor_scan(S[:], psC[:], zer[:], 0.0,
                                             op0=ALU.add, op1=ALU.add)
                nc.vector.tensor_tensor(S[:], S[:], psC[:], op=ALU.subtract)
                psT = ps_g.tile([128, NT], F32, tag="pg")
                nc.tensor.matmul(psT[:], ltmask_b[:], eq[:], start=True, stop=True)
                nc.vector.tensor_tensor(rank[:], S[:], psT[:], op=ALU.add)
                nc.vector.tensor_scalar(tie[:], rank[:], r[:], None, op0=ALU.is_lt)
                nc.vector.tensor_tensor(tie[:], tie[:], eq[:], op=ALU.mult)
                a = a_t[j]
                nc.vector.tensor_tensor(a[:], mgt[:], tie[:], op=ALU.add)
                if j != 1:
                    nc.vector.copy_predicated(u[:], a[:].bitcast(I32), zer[:])
                deferred.append(lambda a=a, j=j: (
                    nc.vector.scalar_tensor_tensor(tm[:], a[:], float(j), tm[:],
                                                   op0=ALU.mult, op1=ALU.add),
                    nc.vector.copy_predicated(ep[:], a[:].bitcast(I32),
                                              probs[:, j::4])))
            for f in deferred:
                f()

            # ---- outputs ----
            ptm = ps_g.tile([NT, 128], F32, tag="pg")
            nc.tensor.transpose(ptm[:], tm[:], ident[:])
            tm_out = misc.tile([NT, 128], I32)
            nc.vector.tensor_copy(tm_out[:], ptm[:])
            nc.sync.dma_start(tm_d[:], tm_out[:])
            pep = ps_g.tile([NT, 128], F32, tag="pg")
            nc.tensor.transpose(pep[:], ep[:], ident[:])
            ep_out = misc.tile([NT, 128], F32)
            nc.vector.tensor_copy(ep_out[:], pep[:])
            nc.scalar.dma_start(ep_d[:], ep_out[:])

    nc.compile()
    return nc


def kernel(input_tokens, W, b):
    from concourse import bass_utils

    if "nc" not in _CACHE:
        _CACHE["nc"] = _build()
    nc = _CACHE["nc"]

    x = np.ascontiguousarray(np.asarray(input_tokens, dtype=np.float32))
    Wf = np.ascontiguousarray(np.asarray(W, dtype=np.float32))
    bf = np.ascontiguousarray(np.asarray(b, dtype=np.float32)).reshape(1, E)
    in_maps = [{"x": x[i], "w": Wf, "b": bf} for i in range(B)]

    trace = bool(int(os.environ.get("CC_TRACE", "0")))
    res = bass_utils.run_bass_kernel_spmd(nc, in_maps, core_ids=list(range(B)),
                                          trace=trace)
    LAST_RUN["exec_time_ns"] = res.exec_time_ns
    LAST_RUN["trace"] = res.instructions_and_trace

    token_mask = np.stack([res.results[i]["tm"].reshape(N) for i in range(B)])
    expert_probs = np.stack([res.results[i]["ep"].reshape(N) for i in range(B)])
    return token_mask.astype(np.int32), expert_probs.astype(np.float32)
